# revision 1
# baseline (speedup 1.0000x reference)
"""Raw-Bacc v3: quarter-grained pipeline, DVE-only v computation,
loads split across both HWDGE queues, consts packed into one DMA.

out[n, c] = pf[c, n] + (Wv @ age + bv)[c]

wvx host-packed [128, 129]: cols 0:64 = Wv, 64:128 = age replicated to
every row, 128 = bv. v = reduce_sum(Wv * age_bc, free) + bv on VectorE
(no PE round-trip for the v chain).
"""

import numpy as np

N_CORES = 8
B, C, D, H, W = 1, 128, 16, 32, 32
N = D * H * W
NSH = N // N_CORES       # 2048
AGE = 64
QTR = 512                # quarter width
NQ = NSH // QTR          # 4


def build_nc():
    import concourse.bacc as bacc
    import concourse.mybir as mybir
    from contextlib import ExitStack

    f32 = mybir.dt.float32
    nc = bacc.Bacc(
        "TRN2", target_bir_lowering=False, debug=False, num_devices=N_CORES)
    pf = nc.dram_tensor("pf", [C, NSH], f32, kind="ExternalInput")
    wvx = nc.dram_tensor("wvx", [C, 2 * AGE + 1], f32, kind="ExternalInput")
    iden = nc.dram_tensor("iden", [128, 128], f32, kind="ExternalInput")
    out = nc.dram_tensor("out", [NSH, C], f32, kind="ExternalOutput")

    # out rows grouped [half h][quarter-in-half g][block j][partition p]
    outv = out.rearrange("(h g j p) c -> h p g j c", p=128, j=QTR // 128,
                         g=2)

    with ExitStack() as ctx:
        e = ctx.enter_context
        sid = e(nc.semaphore("sid"))
        swx = e(nc.semaphore("swx"))
        spf = [e(nc.semaphore(f"spf{q}")) for q in range(NQ)]
        sout = e(nc.semaphore("sout"))
        spe = e(nc.semaphore("spe"))
        sact = e(nc.semaphore("sact"))
        sv = e(nc.semaphore("sv"))
        svc = e(nc.semaphore("svc"))
        identsb = e(nc.sbuf_tensor("identsb", [128, 128], f32))
        wvxsb = e(nc.sbuf_tensor("wvxsb", [C, 2 * AGE + 1], f32))
        tmp = e(nc.sbuf_tensor("tmp", [C, AGE], f32))
        vsum = e(nc.sbuf_tensor("vsum", [C, 1], f32))
        vcol = e(nc.sbuf_tensor("vcol", [C, 1], f32))
        pft = e(nc.sbuf_tensor("pft", [C, NSH], f32))
        osb0 = e(nc.sbuf_tensor("osb0", [128, 2 * QTR], f32))
        osb1 = e(nc.sbuf_tensor("osb1", [128, 2 * QTR], f32))
        pgs = [e(nc.psum_tensor(f"pg{q}", [128, QTR], f32)) for q in range(NQ)]
        block = e(nc.Block())
        osbs = [osb0, osb1]

        @block.sync
        def _(sync):
            sync.dma_start(out=identsb[:], in_=iden[:]).then_inc(sid, 16)
            sync.dma_start(
                out=pft[:, 0 * QTR:1 * QTR],
                in_=pf[:, 0 * QTR:1 * QTR]).then_inc(spf[0], 16)
            sync.dma_start(
                out=pft[:, 2 * QTR:3 * QTR],
                in_=pf[:, 2 * QTR:3 * QTR]).then_inc(spf[2], 16)
            sync.wait_ge(svc, 2)
            sync.dma_start(
                out=outv[0],
                in_=osb0[:].rearrange("p (g j c) -> p g j c", c=128,
                                      j=QTR // 128),
            ).then_inc(sout, 16)
            sync.wait_ge(sout, 32)

        @block.scalar
        def _(scalar):
            import concourse.mybir as mybir

            scalar.dma_start(out=wvxsb[:], in_=wvx[:]).then_inc(swx, 16)
            scalar.dma_start(
                out=pft[:, 1 * QTR:2 * QTR],
                in_=pf[:, 1 * QTR:2 * QTR]).then_inc(spf[1], 16)
            scalar.dma_start(
                out=pft[:, 3 * QTR:4 * QTR],
                in_=pf[:, 3 * QTR:4 * QTR]).then_inc(spf[3], 16)
            scalar.wait_ge(sv, 1)
            for q in range(NQ):
                scalar.wait_ge(spf[q], 16)
                scalar.activation(
                    pft[:, q * QTR:(q + 1) * QTR],
                    pft[:, q * QTR:(q + 1) * QTR],
                    mybir.ActivationFunctionType.Identity,
                    bias=vcol[:],
                ).then_inc(sact, 1)
            scalar.wait_ge(svc, 4)
            scalar.dma_start(
                out=outv[1],
                in_=osb1[:].rearrange("p (g j c) -> p g j c", c=128,
                                      j=QTR // 128),
            ).then_inc(sout, 16)

        @block.tensor
        def _(tensor):
            tensor.wait_ge(sid, 16)
            for q in range(NQ):
                tensor.wait_ge(sact, q + 1)
                for j in range(QTR // 128):
                    c0 = q * QTR + j * 128
                    ins = tensor.transpose(
                        pgs[q][:, j * 128:(j + 1) * 128],
                        pft[:, c0:c0 + 128],
                        identsb[:],
                    )
                    if j == QTR // 128 - 1:
                        ins.then_inc(spe, 1)

        @block.vector
        def _(vector):
            import concourse.mybir as mybir

            vector.wait_ge(swx, 16)
            vector.tensor_tensor(
                tmp[:], wvxsb[:, 0:AGE], wvxsb[:, AGE:2 * AGE],
                mybir.AluOpType.mult)
            vector.reduce_sum(vsum[:], tmp[:], axis=mybir.AxisListType.X)
            vector.tensor_scalar(
                out=vcol[:], in0=vsum[:],
                scalar1=wvxsb[:, 2 * AGE:2 * AGE + 1], scalar2=None,
                op0=mybir.AluOpType.add,
            ).then_inc(sv, 1)
            for q in range(NQ):
                vector.wait_ge(spe, q + 1)
                vector.tensor_copy(
                    osbs[q // 2][:, (q % 2) * QTR:(q % 2 + 1) * QTR],
                    pgs[q][:],
                ).then_inc(svc, 1)

    nc.finalize()
    return nc


_CACHE = {}
LAST_RESULTS = None


def kernel(**inputs):
    global LAST_RESULTS
    from concourse.bass_utils import run_bass_kernel_spmd

    if "nc" not in _CACHE:
        _CACHE["nc"] = build_nc()
    nc = _CACHE["nc"]

    pf_full = np.ascontiguousarray(
        np.asarray(inputs["pixel_features"], dtype=np.float32).reshape(C, N))
    age = np.asarray(inputs["age_features"], dtype=np.float32).reshape(AGE)
    wvx_np = np.empty((C, 2 * AGE + 1), dtype=np.float32)
    wvx_np[:, 0:AGE] = np.asarray(inputs["Wv"], dtype=np.float32)
    wvx_np[:, AGE:2 * AGE] = age[None, :]
    wvx_np[:, 2 * AGE] = np.asarray(inputs["bv"], dtype=np.float32)
    iden_np = np.eye(128, dtype=np.float32)

    in_maps = [
        {
            "pf": np.ascontiguousarray(pf_full[:, i * NSH:(i + 1) * NSH]),
            "wvx": wvx_np,
            "iden": iden_np,
        }
        for i in range(N_CORES)
    ]
    res = run_bass_kernel_spmd(nc, in_maps, core_ids=list(range(N_CORES)))
    LAST_RESULTS = res
    out = np.concatenate([res.results[i]["out"] for i in range(N_CORES)], axis=0)
    return out.reshape(B, N, C).astype(np.float32)



# revision 12
# speedup vs baseline: 1.0732x; 1.0732x over previous
"""Raw-Bacc v5: per-128-block pipeline, fused +v via DVE tensor_tensor,
per-quarter output DMAs overlapped with loads.

out[n, c] = pf[c, n] + v[c],  v = Wv @ age + bv

wvx host-packed [128, 129] f32 (baseline layout):
  cols  0:64  = Wv
  cols 64:128 = age replicated to every row
  col  128    = bv
v chain: DVE  vcol = reduce_sum(Wv*age_bc) + bv        [128, 1]
         ACT  vb1[p, f] = vcol[p]   (scale=0, bias)    [128, 128]
         PE   pgv = vb1.T  -> vbc[p, c] = v[c]         [128, 128]
         DVE  vbc <- copy(pgv)
Per 128-col block j of quarter q: PE transpose pft block -> psum,
DVE tensor_tensor(osb = psum + vbc). Out DMA per quarter issued as
soon as its 4 blocks land in SBUF; loads split across both HWDGE rings.
"""

import numpy as np

N_CORES = 8
B, C, D, H, W = 1, 128, 16, 32, 32
N = D * H * W
NSH = N // N_CORES       # 2048
AGE = 64
QTR = 512                # quarter width (cols per out DMA)
NQ = NSH // QTR          # 4
JB = QTR // 128          # 128-col blocks per quarter


def build_nc():
    import concourse.bacc as bacc
    import concourse.mybir as mybir
    from contextlib import ExitStack

    f32 = mybir.dt.float32
    nc = bacc.Bacc(
        "TRN2", target_bir_lowering=False, debug=False, num_devices=N_CORES)
    pf = nc.dram_tensor("pf", [C, NSH], f32, kind="ExternalInput")
    wvx = nc.dram_tensor("wvx", [C, 2 * AGE + 1], f32, kind="ExternalInput")
    iden = nc.dram_tensor("iden", [128, 128], f32, kind="ExternalInput")
    out = nc.dram_tensor("out", [NSH, C], f32, kind="ExternalOutput")

    # out rows grouped [quarter q][block j][partition p] -> view [q, p, j, c]
    outv = out.rearrange("(q j p) c -> q p j c", p=128, j=JB)

    with ExitStack() as ctx:
        e = ctx.enter_context
        swx = e(nc.semaphore("swx"))
        spf = [e(nc.semaphore(f"spf{q}")) for q in range(NQ)]
        sidv = e(nc.semaphore("sidv"))
        sv1 = e(nc.semaphore("sv1"))
        sv2 = e(nc.semaphore("sv2"))
        svcol = e(nc.semaphore("svcol"))
        svb1 = e(nc.semaphore("svb1"))
        svb = e(nc.semaphore("svb"))
        svbc = e(nc.semaphore("svbc"))
        stp = [e(nc.semaphore(f"stp{q}")) for q in range(NQ)]
        scp = [e(nc.semaphore(f"scp{q}")) for q in range(NQ)]
        sout = e(nc.semaphore("sout"))

        wvxsb = e(nc.sbuf_tensor("wvxsb", [C, 2 * AGE + 1], f32))
        identsb = e(nc.sbuf_tensor("identsb", [128, 128], f32))
        tmp = e(nc.sbuf_tensor("tmp", [C, AGE], f32))
        vsum = e(nc.sbuf_tensor("vsum", [C, 1], f32))
        vcol = e(nc.sbuf_tensor("vcol", [C, 1], f32))
        vb1 = e(nc.sbuf_tensor("vb1", [128, 128], f32))
        vbc = e(nc.sbuf_tensor("vbc", [128, 128], f32))
        pft = e(nc.sbuf_tensor("pft", [C, NSH], f32))
        osb = [e(nc.sbuf_tensor(f"osb{q}", [128, QTR], f32))
               for q in range(NQ)]
        pgv = e(nc.psum_tensor("pgv", [128, QTR], f32))
        pgs = [e(nc.psum_tensor(f"pg{q}", [128, QTR], f32))
               for q in range(NQ)]
        block = e(nc.Block())

        @block.sync
        def _(sync):
            sync.dma_start(
                out=pft[:, 0 * QTR:1 * QTR],
                in_=pf[:, 0 * QTR:1 * QTR]).then_inc(spf[0], 16)
            sync.dma_start(out=identsb[:], in_=iden[:]).then_inc(sidv, 16)
            sync.dma_start(
                out=pft[:, 2 * QTR:3 * QTR],
                in_=pf[:, 2 * QTR:3 * QTR]).then_inc(spf[2], 16)
            for q in (0, 2):
                sync.wait_ge(scp[q], 1)
                sync.dma_start(
                    out=outv[q],
                    in_=osb[q][:].rearrange("p (j c) -> p j c", c=128),
                ).then_inc(sout, 16)
            sync.wait_ge(sout, 16 * 4)

        @block.scalar
        def _(scalar):
            import concourse.mybir as mybir

            scalar.dma_start(out=wvxsb[:], in_=wvx[:]).then_inc(swx, 16)
            scalar.dma_start(
                out=pft[:, 1 * QTR:2 * QTR],
                in_=pf[:, 1 * QTR:2 * QTR]).then_inc(spf[1], 16)
            scalar.dma_start(
                out=pft[:, 3 * QTR:4 * QTR],
                in_=pf[:, 3 * QTR:4 * QTR]).then_inc(spf[3], 16)
            scalar.wait_ge(svcol, 1)
            scalar.activation(
                vb1[:], wvxsb[:, 0:128],
                mybir.ActivationFunctionType.Identity,
                bias=vcol[:], scale=0.0,
            ).then_inc(svb1, 1)
            for q in (1, 3):
                scalar.wait_ge(scp[q], 1)
                scalar.dma_start(
                    out=outv[q],
                    in_=osb[q][:].rearrange("p (j c) -> p j c", c=128),
                ).then_inc(sout, 16)

        @block.tensor
        def _(tensor):
            tensor.wait_ge(sidv, 16)
            tensor.wait_ge(svb1, 1)
            tensor.transpose(
                pgv[:, 0:128], vb1[:], identsb[:]).then_inc(svb, 1)
            for q in range(NQ):
                tensor.wait_ge(spf[q], 16)
                for j in range(JB):
                    c0 = q * QTR + j * 128
                    tensor.transpose(
                        pgs[q][:, j * 128:(j + 1) * 128],
                        pft[:, c0:c0 + 128],
                        identsb[:],
                    ).then_inc(stp[q], 1)

        @block.vector
        def _(vector):
            import concourse.mybir as mybir

            vector.wait_ge(swx, 16)
            vector.tensor_tensor(
                tmp[:], wvxsb[:, 0:AGE], wvxsb[:, AGE:2 * AGE],
                mybir.AluOpType.mult).then_inc(sv1, 1)
            vector.wait_ge(sv1, 1)
            vector.reduce_sum(
                vsum[:], tmp[:], axis=mybir.AxisListType.X).then_inc(sv2, 1)
            vector.wait_ge(sv2, 1)
            vector.tensor_scalar(
                out=vcol[:], in0=vsum[:],
                scalar1=wvxsb[:, 2 * AGE:2 * AGE + 1], scalar2=None,
                op0=mybir.AluOpType.add,
            ).then_inc(svcol, 1)
            vector.wait_ge(svb, 1)
            vector.tensor_copy(vbc[:], pgv[:, 0:128]).then_inc(svbc, 1)
            vector.wait_ge(svbc, 1)
            for q in range(NQ):
                for j in range(JB):
                    vector.wait_ge(stp[q], j + 1)
                    ins = vector.tensor_tensor(
                        osb[q][:, j * 128:(j + 1) * 128],
                        pgs[q][:, j * 128:(j + 1) * 128],
                        vbc[:],
                        mybir.AluOpType.add)
                    if j == JB - 1:
                        ins.then_inc(scp[q], 1)

    nc.finalize()
    return nc


_CACHE = {}
LAST_RESULTS = None


def kernel(**inputs):
    global LAST_RESULTS
    from concourse.bass_utils import run_bass_kernel_spmd

    if "nc" not in _CACHE:
        _CACHE["nc"] = build_nc()
    nc = _CACHE["nc"]

    pf_full = np.ascontiguousarray(
        np.asarray(inputs["pixel_features"], dtype=np.float32).reshape(C, N))
    age = np.asarray(inputs["age_features"], dtype=np.float32).reshape(AGE)
    wvx_np = np.empty((C, 2 * AGE + 1), dtype=np.float32)
    wvx_np[:, 0:AGE] = np.asarray(inputs["Wv"], dtype=np.float32)
    wvx_np[:, AGE:2 * AGE] = age[None, :]
    wvx_np[:, 2 * AGE] = np.asarray(inputs["bv"], dtype=np.float32)
    iden_np = np.eye(128, dtype=np.float32)

    in_maps = [
        {
            "pf": np.ascontiguousarray(pf_full[:, i * NSH:(i + 1) * NSH]),
            "wvx": wvx_np,
            "iden": iden_np,
        }
        for i in range(N_CORES)
    ]
    res = run_bass_kernel_spmd(nc, in_maps, core_ids=list(range(N_CORES)))
    LAST_RESULTS = res
    out = np.concatenate([res.results[i]["out"] for i in range(N_CORES)], axis=0)
    return out.reshape(B, N, C).astype(np.float32)


# revision 13
# speedup vs baseline: 1.0896x; 1.0153x over previous
"""Raw-Bacc v6: quarter-grained pipeline with fused +v on DVE and
per-quarter output DMAs overlapped with remaining loads/compute.

out[n, c] = pf[c, n] + v[c],  v = Wv @ age + bv

wvx host-packed [128, 129] f32:
  cols  0:64  = Wv, cols 64:128 = age bcast, col 128 = bv
v chain: DVE  vcol = reduce_sum(Wv*age_bc) + bv        [128, 1]
         ACT  vb1[p, f] = vcol[p]   (scale=0, bias)    [128, 128]
         PE   pgv = vb1.T  (vbc row-broadcast of v)    psum [128, 128]
         DVE  vbc4 <- 4x copy(pgv)                     [128, 512]
Per quarter q: PE 4x 128-transpose -> psum bank q,
DVE tensor_tensor(osb_q = pg_q + vbc4), out DMA issued immediately.
Loads split across both HWDGE rings; out DMAs alternate rings.
"""

import numpy as np

N_CORES = 8
B, C, D, H, W = 1, 128, 16, 32, 32
N = D * H * W
NSH = N // N_CORES       # 2048
AGE = 64
QTR = 512                # quarter width (cols per out DMA)
NQ = NSH // QTR          # 4
JB = QTR // 128          # 128-col blocks per quarter


def build_nc():
    import concourse.bacc as bacc
    import concourse.mybir as mybir
    from contextlib import ExitStack

    f32 = mybir.dt.float32
    nc = bacc.Bacc(
        "TRN2", target_bir_lowering=False, debug=False, num_devices=N_CORES)
    pf = nc.dram_tensor("pf", [C, NSH], f32, kind="ExternalInput")
    wvx = nc.dram_tensor("wvx", [C, 2 * AGE + 1], f32, kind="ExternalInput")
    iden = nc.dram_tensor("iden", [128, 128], f32, kind="ExternalInput")
    out = nc.dram_tensor("out", [NSH, C], f32, kind="ExternalOutput")

    outv = out.rearrange("(q j p) c -> q p j c", p=128, j=JB)

    with ExitStack() as ctx:
        e = ctx.enter_context
        sid = e(nc.semaphore("sid"))
        swx = e(nc.semaphore("swx"))
        spf = [e(nc.semaphore(f"spf{q}")) for q in range(NQ)]
        sv1 = e(nc.semaphore("sv1"))
        sv2 = e(nc.semaphore("sv2"))
        svcol = e(nc.semaphore("svcol"))
        svb1 = e(nc.semaphore("svb1"))
        svb = e(nc.semaphore("svb"))
        svbc = e(nc.semaphore("svbc"))
        spe = e(nc.semaphore("spe"))
        svc = e(nc.semaphore("svc"))
        sout = e(nc.semaphore("sout"))

        identsb = e(nc.sbuf_tensor("identsb", [128, 128], f32))
        wvxsb = e(nc.sbuf_tensor("wvxsb", [C, 2 * AGE + 1], f32))
        tmp = e(nc.sbuf_tensor("tmp", [C, AGE], f32))
        vsum = e(nc.sbuf_tensor("vsum", [C, 1], f32))
        vcol = e(nc.sbuf_tensor("vcol", [C, 1], f32))
        vb1 = e(nc.sbuf_tensor("vb1", [128, 128], f32))
        vbc4 = e(nc.sbuf_tensor("vbc4", [128, QTR], f32))
        pft = e(nc.sbuf_tensor("pft", [C, NSH], f32))
        osb = [e(nc.sbuf_tensor(f"osb{q}", [128, QTR], f32))
               for q in range(NQ)]
        pgv = e(nc.psum_tensor("pgv", [128, QTR], f32))
        pgs = [e(nc.psum_tensor(f"pg{q}", [128, QTR], f32))
               for q in range(NQ)]
        block = e(nc.Block())

        @block.sync
        def _(sync):
            sync.dma_start(out=identsb[:], in_=iden[:]).then_inc(sid, 16)
            sync.dma_start(
                out=pft[:, 0 * QTR:1 * QTR],
                in_=pf[:, 0 * QTR:1 * QTR]).then_inc(spf[0], 16)
            sync.dma_start(
                out=pft[:, 2 * QTR:3 * QTR],
                in_=pf[:, 2 * QTR:3 * QTR]).then_inc(spf[2], 16)
            for q in (0, 2):
                sync.wait_ge(svc, q + 1)
                sync.dma_start(
                    out=outv[q],
                    in_=osb[q][:].rearrange("p (j c) -> p j c", c=128),
                ).then_inc(sout, 16)
            sync.wait_ge(sout, 64)

        @block.scalar
        def _(scalar):
            import concourse.mybir as mybir

            scalar.dma_start(out=wvxsb[:], in_=wvx[:]).then_inc(swx, 16)
            scalar.dma_start(
                out=pft[:, 1 * QTR:2 * QTR],
                in_=pf[:, 1 * QTR:2 * QTR]).then_inc(spf[1], 16)
            scalar.dma_start(
                out=pft[:, 3 * QTR:4 * QTR],
                in_=pf[:, 3 * QTR:4 * QTR]).then_inc(spf[3], 16)
            scalar.wait_ge(svcol, 1)
            scalar.activation(
                vb1[:], wvxsb[:, 0:128],
                mybir.ActivationFunctionType.Identity,
                bias=vcol[:], scale=0.0,
            ).then_inc(svb1, 1)
            for q in (1, 3):
                scalar.wait_ge(svc, q + 1)
                scalar.dma_start(
                    out=outv[q],
                    in_=osb[q][:].rearrange("p (j c) -> p j c", c=128),
                ).then_inc(sout, 16)

        @block.tensor
        def _(tensor):
            tensor.wait_ge(sid, 16)
            tensor.wait_ge(svb1, 1)
            tensor.transpose(
                pgv[:, 0:128], vb1[:], identsb[:]).then_inc(svb, 1)
            for q in range(NQ):
                tensor.wait_ge(spf[q], 16)
                for j in range(JB):
                    c0 = q * QTR + j * 128
                    ins = tensor.transpose(
                        pgs[q][:, j * 128:(j + 1) * 128],
                        pft[:, c0:c0 + 128],
                        identsb[:],
                    )
                    if j == JB - 1:
                        ins.then_inc(spe, 1)

        @block.vector
        def _(vector):
            import concourse.mybir as mybir

            vector.wait_ge(swx, 16)
            vector.tensor_tensor(
                tmp[:], wvxsb[:, 0:AGE], wvxsb[:, AGE:2 * AGE],
                mybir.AluOpType.mult).then_inc(sv1, 1)
            vector.wait_ge(sv1, 1)
            vector.reduce_sum(
                vsum[:], tmp[:], axis=mybir.AxisListType.X).then_inc(sv2, 1)
            vector.wait_ge(sv2, 1)
            vector.tensor_scalar(
                out=vcol[:], in0=vsum[:],
                scalar1=wvxsb[:, 2 * AGE:2 * AGE + 1], scalar2=None,
                op0=mybir.AluOpType.add,
            ).then_inc(svcol, 1)
            vector.wait_ge(svb, 1)
            for j in range(JB):
                ins = vector.tensor_copy(
                    vbc4[:, j * 128:(j + 1) * 128], pgv[:, 0:128])
                if j == JB - 1:
                    ins.then_inc(svbc, 1)
            vector.wait_ge(svbc, 1)
            for q in range(NQ):
                vector.wait_ge(spe, q + 1)
                vector.tensor_tensor(
                    osb[q][:], pgs[q][:], vbc4[:],
                    mybir.AluOpType.add).then_inc(svc, 1)

    nc.finalize()
    return nc


_CACHE = {}
LAST_RESULTS = None


def kernel(**inputs):
    global LAST_RESULTS
    from concourse.bass_utils import run_bass_kernel_spmd

    if "nc" not in _CACHE:
        _CACHE["nc"] = build_nc()
    nc = _CACHE["nc"]

    pf_full = np.ascontiguousarray(
        np.asarray(inputs["pixel_features"], dtype=np.float32).reshape(C, N))
    age = np.asarray(inputs["age_features"], dtype=np.float32).reshape(AGE)
    wvx_np = np.empty((C, 2 * AGE + 1), dtype=np.float32)
    wvx_np[:, 0:AGE] = np.asarray(inputs["Wv"], dtype=np.float32)
    wvx_np[:, AGE:2 * AGE] = age[None, :]
    wvx_np[:, 2 * AGE] = np.asarray(inputs["bv"], dtype=np.float32)
    iden_np = np.eye(128, dtype=np.float32)

    in_maps = [
        {
            "pf": np.ascontiguousarray(pf_full[:, i * NSH:(i + 1) * NSH]),
            "wvx": wvx_np,
            "iden": iden_np,
        }
        for i in range(N_CORES)
    ]
    res = run_bass_kernel_spmd(nc, in_maps, core_ids=list(range(N_CORES)))
    LAST_RESULTS = res
    out = np.concatenate([res.results[i]["out"] for i in range(N_CORES)], axis=0)
    return out.reshape(B, N, C).astype(np.float32)
